# revision 1
# baseline (speedup 1.0000x reference)
"""Trainium2 Bass kernel for multi-head attention (GQA + RoPE + causal).

Problem shapes (hardcoded):
  x: (2, 2048, 2048)  Wq: (2048, 2048->512/core)  Wk/Wv: (2048, 512->128/core)
  Wo: (2048->512/core, 2048)  cos/sin: (2048, 64)  mask: causal (1,1,2048,2048)

Sharding: 8 cores = 2 batches (DP) x 4 head groups (TP).  Each core handles
one batch sample and 8 query heads (= 2 KV heads, keeping each KV head with
its 4 query heads).  Wo's input dim is sharded, so each core produces a
partial (2048, 2048) output; the host sums the 4 partials per batch.

Per-core kernel strategy (all matmuls in fp16 — same 10-bit mantissa as
TF32/fp32r but with fast-weight-load and half the DMA bytes):
  - QKV projections computed TRANSPOSED: Q^T[do,s] = Wq[din,do].T @ x^T[din,s]
    (x is pre-transposed on the host so x^T tiles DMA contiguously).
  - RoPE applied in-place on Q^T/K^T via partition-shifted SBUF copies and
    host-preprocessed cos/sin tables (transposed, duplicated, sign-folded).
  - scores computed transposed per head: S^T[k,q] = K^T.T @ Q^T with k-tiles
    of 128 and q-blocks of 512; fully-masked tiles skipped (causal), diagonal
    tiles zeroed post-exp with gpsimd.affine_select.
  - softmax without max-subtraction (scores are O(10), exp is safe in fp32);
    exp on the scalar engine with the 1/sqrt(64) scale folded in.
  - PV matmul O~^T[d,q] = [V|1].T @ P^T accumulated over k-tiles in PSUM; the
    appended ones-column makes row 64 the softmax denominator for free.
  - normalize with vector.reciprocal + gpsimd.partition_broadcast, writing
    the normalized attention output transposed (attnT[head_dim*8, seq]).
  - output projection out[s,dm] = attnT[:,s_tile].T @ Wo chunks, PSUM
    accumulated over the 4 hd-chunks, written back as a partial result.
"""

import os
import sys

import numpy as np

if "/opt/trn_rl_repo" not in sys.path:
    sys.path.insert(0, "/opt/trn_rl_repo")

SEQ = 2048
DIM = 2048
HEAD_DIM = 64
N_HEADS_CORE = 8  # query heads per core
DQ = N_HEADS_CORE * HEAD_DIM  # 512
DKV = 2 * HEAD_DIM  # 128 (2 kv heads per core)
SCALE = HEAD_DIM ** -0.5
N_CORES = 8
F32 = None  # set after import

_PROGRAM_CACHE = {}


def _build_program(causal: bool):
    import concourse.bass as bass  # noqa: F401
    import concourse.mybir as mybir
    from concourse import bacc
    from concourse.masks import make_identity
    from concourse.tile import TileContext

    f32 = mybir.dt.float32
    f16 = mybir.dt.float16
    AOT = mybir.AluOpType

    nc = bacc.Bacc(None, target_bir_lowering=False)
    xT = nc.declare_dram_parameter("xT", [DIM, SEQ], f16, isOutput=False)
    wq = nc.declare_dram_parameter("wq", [DIM, DQ], f16, isOutput=False)
    wk = nc.declare_dram_parameter("wk", [DIM, DKV], f16, isOutput=False)
    wv = nc.declare_dram_parameter("wv", [DIM, DKV], f16, isOutput=False)
    wo = nc.declare_dram_parameter("wo", [DQ, DIM], f16, isOutput=False)
    cos2 = nc.declare_dram_parameter("cos2", [128, SEQ], f16, isOutput=False)
    sin2 = nc.declare_dram_parameter("sin2", [128, SEQ], f16, isOutput=False)
    out = nc.declare_dram_parameter("out", [SEQ, DIM], f32, isOutput=True)

    NSEQT = SEQ // 128  # 16 k-tiles / s-tiles
    NQB = SEQ // 512  # 4 q/seq blocks
    NDIN = DIM // 128  # 16 contraction chunks
    GRP = 2  # k-tiles per exp batch in phase 2

    with TileContext(nc) as tc:
        with tc.tile_pool(name="persist", bufs=1) as pa, \
             tc.tile_pool(name="ph1", bufs=1) as pb, \
             tc.tile_pool(name="ph2", bufs=1) as pc:
            # per-j [128, 512] tiles keep Tile deps exact (no false
            # serialization on seq-wide tensors)
            qt = [[pa.tile([128, 512], f16, name=f"qt{t}_{j}",
                           tag=f"qt{t}_{j}") for j in range(NQB)]
                  for t in range(4)]
            ktr = [pa.tile([128, 512], f16, name=f"ktr{j}", tag=f"ktr{j}")
                   for j in range(NQB)]
            kdup = [[pa.tile([128, 512], f16, name=f"kdup{g}_{j}",
                             tag=f"kdup{g}_{j}") for j in range(NQB)]
                    for g in range(2)]
            vtiles = [pa.tile([128, 130], f16, name=f"vt{i}", tag=f"vt{i}")
                      for i in range(NSEQT)]
            attnT = [[pa.tile([128, 512], f16, name=f"attnT{t}_{j}",
                              tag=f"attnT{t}_{j}") for j in range(NQB)]
                     for t in range(4)]
            identity = pa.tile([128, 128], f16, name="identity",
                               tag="identity")
            make_identity(nc, identity)
            cos_sb = pa.tile([128, SEQ], f16, name="cos_sb", tag="cos_sb")
            sin_sb = pa.tile([128, SEQ], f16, name="sin_sb", tag="sin_sb")

            # ---------------- phase 1: QKV projections + RoPE --------------
            with tc.tile_pool(name="ph1ps", bufs=1, space="PSUM") as pbps:
                wq_sb = pb.tile([128, NDIN, DQ], f16, name="wq_sb",
                                tag="wq_sb")
                wk_sb = pb.tile([128, NDIN, DKV], f16, name="wk_sb",
                                tag="wk_sb")
                wv_sb = pb.tile([128, NDIN, DKV], f16, name="wv_sb",
                                tag="wv_sb")
                for c in range(NDIN):
                    nc.scalar.dma_start(out=wq_sb[:, c, :],
                                        in_=wq[c * 128:(c + 1) * 128, :])
                    nc.scalar.dma_start(out=wk_sb[:, c, :],
                                        in_=wk[c * 128:(c + 1) * 128, :])
                    nc.scalar.dma_start(out=wv_sb[:, c, :],
                                        in_=wv[c * 128:(c + 1) * 128, :])
                nc.scalar.dma_start(out=cos_sb, in_=cos2[:, :])
                nc.scalar.dma_start(out=sin_sb, in_=sin2[:, :])

                vtrs = []
                for j in range(NQB):
                    sl = slice(j * 512, (j + 1) * 512)
                    qps = [pbps.tile([128, 512], f32, name=f"qps{t}",
                                     tag=f"qps{t}") for t in range(4)]
                    kps = pbps.tile([128, 512], f32, name="kps", tag="kps")
                    vps = pbps.tile([128, 512], f32, name="vps", tag="vps")
                    for c in range(NDIN):
                        xt = pb.tile([128, 512], f16, name="xt", tag="xt",
                                     bufs=6)
                        nc.sync.dma_start(out=xt, in_=xT[c * 128:(c + 1) * 128,
                                                         sl])
                        st, sp = (c == 0), (c == NDIN - 1)
                        for t in range(4):
                            nc.tensor.matmul(
                                qps[t],
                                lhsT=wq_sb[:, c, t * 128:(t + 1) * 128],
                                rhs=xt, start=st, stop=sp)
                        nc.tensor.matmul(kps, lhsT=wk_sb[:, c, :], rhs=xt,
                                         start=st, stop=sp)
                        nc.tensor.matmul(vps, lhsT=wv_sb[:, c, :], rhs=xt,
                                         start=st, stop=sp)
                    vtr = pb.tile([128, 512], f16, name="vtr", tag="vtr",
                                  bufs=4)
                    vtrs.append(vtr)
                    nc.scalar.copy(out=ktr[j], in_=kps)
                    for t in range(3):
                        nc.scalar.copy(out=qt[t][j], in_=qps[t])
                    nc.vector.tensor_copy(out=qt[3][j], in_=qps[3])
                    nc.vector.tensor_copy(out=vtr, in_=vps)
                    # RoPE (in place) on this j-slice of K^T then Q^T chunks
                    for chunk in [ktr[j]] + [qt[t][j] for t in range(4)]:
                        rot = pb.tile([128, 512], f16, name="rot", tag="rot",
                                      bufs=3)
                        for blk in (0, 64):
                            nc.scalar.dma_start(
                                out=rot[blk:blk + 32, :],
                                in_=chunk[blk + 32:blk + 64, :])
                            nc.scalar.dma_start(
                                out=rot[blk + 32:blk + 64, :],
                                in_=chunk[blk:blk + 32, :])
                        nc.vector.tensor_tensor(out=rot, in0=rot,
                                                in1=sin_sb[:, sl],
                                                op=AOT.mult)
                        nc.vector.tensor_tensor(out=chunk, in0=chunk,
                                                in1=cos_sb[:, sl],
                                                op=AOT.mult)
                        nc.vector.tensor_add(out=chunk, in0=chunk, in1=rot)
                    # kv-head dups
                    nc.gpsimd.dma_start(out=kdup[0][j][0:64, :],
                                        in_=ktr[j][0:64, :])
                    nc.gpsimd.dma_start(out=kdup[0][j][64:128, :],
                                        in_=ktr[j][0:64, :])
                    nc.gpsimd.dma_start(out=kdup[1][j][0:64, :],
                                        in_=ktr[j][64:128, :])
                    nc.gpsimd.dma_start(out=kdup[1][j][64:128, :],
                                        in_=ktr[j][64:128, :])

            # ------------- phase 2+3: attention + output projection --------
            with tc.tile_pool(name="ph2ps", bufs=1, space="PSUM") as pcps:
                wo_sb = [pc.tile([128, DIM], f16, name=f"wo_sb{c}",
                                 tag=f"wo_sb{c}") for c in range(4)]
                for c in range(4):
                    nc.scalar.dma_start(out=wo_sb[c],
                                        in_=wo[c * 128:(c + 1) * 128, :])
                for i in range(NSEQT):
                    vt_ps = pcps.tile([128, 128], f16, name="vt_ps",
                                      tag="pvops", bufs=4)
                    nc.tensor.transpose(
                        vt_ps, vtrs[i // 4][:, (i % 4) * 128:
                                            (i % 4 + 1) * 128], identity)
                    nc.vector.tensor_copy(out=vtiles[i][:, 0:64],
                                          in_=vt_ps[:, 0:64])
                    nc.vector.tensor_copy(out=vtiles[i][:, 65:129],
                                          in_=vt_ps[:, 64:128])
                    nc.vector.memset(vtiles[i][:, 64:65], 1.0)
                    nc.vector.memset(vtiles[i][:, 129:130], 1.0)
                def oproj(j):
                    # output projection for block j's 4 s-tiles
                    for s_ in range(4 * j, 4 * j + 4):
                        so = (s_ - 4 * j) * 128
                        ostage = pc.tile([128, DIM], f32, name="ostage",
                                         tag="ostage", bufs=2)
                        for dm in range(4):
                            ops = pcps.tile([128, 512], f32, name="ops",
                                            tag="pvops", bufs=4)
                            for c in range(4):
                                nc.tensor.matmul(
                                    ops,
                                    lhsT=attnT[c][j][:, so:so + 128],
                                    rhs=wo_sb[c][:, dm * 512:(dm + 1) * 512],
                                    start=(c == 0), stop=(c == 3))
                            nc.vector.tensor_copy(
                                out=ostage[:, dm * 512:(dm + 1) * 512],
                                in_=ops)
                        nc.sync.dma_start(
                            out=out[s_ * 128:(s_ + 1) * 128, :], in_=ostage)

                for j in range(NQB):
                    nkt = 4 * j + 4 if causal else NSEQT
                    for hp in range(4):  # head pairs (2hp, 2hp+1)
                        g = hp // 2       # local kv head (shared by the pair)
                        pv_e = pcps.tile([65, 512], f32, name="pv_e",
                                         tag="pvops", bufs=4)
                        pv_o = pcps.tile([65, 512], f32, name="pv_o",
                                         tag="pvops", bufs=4)
                        if causal:
                            kt_order = list(range(4 * j, nkt)) + \
                                list(range(0, 4 * j))
                        else:
                            kt_order = list(range(nkt))
                        first_kt, last_kt = kt_order[0], kt_order[-1]
                        for kt in kt_order:
                            stt = pcps.tile([128, 2, 512], f32, name="stt",
                                            tag="stt", bufs=2)
                            lk = kdup[g][kt // 4]
                            ck = slice((kt % 4) * 128, (kt % 4 + 1) * 128)
                            nc.tensor.matmul(stt[:, 0, :],
                                             lhsT=lk[0:64, ck],
                                             rhs=qt[hp][j][0:64, :],
                                             start=True, stop=True,
                                             tile_position=(0, 0))
                            nc.tensor.matmul(stt[:, 1, :],
                                             lhsT=lk[64:128, ck],
                                             rhs=qt[hp][j][64:128, :],
                                             start=True, stop=True,
                                             tile_position=(64, 0))
                            pt = pc.tile([128, 2, 512], f16, name="pt",
                                         tag="pt", bufs=6)
                            nc.scalar.activation(
                                out=pt, in_=stt,
                                func=mybir.ActivationFunctionType.Exp,
                                scale=SCALE)
                            if causal and kt >= 4 * j:
                                i = kt - 4 * j
                                nc.gpsimd.affine_select(
                                    out=pt, in_=pt,
                                    pattern=[[0, 2], [1, 512]],
                                    compare_op=AOT.is_ge,
                                    fill=0.0, base=-128 * i,
                                    channel_multiplier=-1)
                            st, sp = (kt == first_kt), (kt == last_kt)
                            nc.tensor.matmul(
                                pv_e, lhsT=vtiles[kt][:, 65 * g:65 * g + 65],
                                rhs=pt[:, 0, :], start=st, stop=sp)
                            nc.tensor.matmul(
                                pv_o, lhsT=vtiles[kt][:, 65 * g:65 * g + 65],
                                rhs=pt[:, 1, :], start=st, stop=sp)
                        for par, pv in ((0, pv_e), (1, pv_o)):
                            # fast drain frees the PSUM slot; normalize off
                            # the critical path (denom row copied to base 0
                            # for the custom-DVE approx reciprocal)
                            pvs = pc.tile([64, 512], f32, name="pvs",
                                          tag="pvs", bufs=4)
                            nc.vector.tensor_copy(out=pvs,
                                                  in_=pv[0:64, :])
                            den = pc.tile([1, 512], f32, name="den",
                                          tag="den", bufs=4)
                            nc.scalar.copy(out=den, in_=pv[64:65, :])
                            rec = pc.tile([1, 512], f32, name="rec",
                                          tag="rec", bufs=4)
                            nc.vector.reciprocal_approx_fast(out=rec, in_=den)
                            rbc = pc.tile([64, 512], f32, name="rbc",
                                          tag="rbc", bufs=4)
                            nc.gpsimd.partition_broadcast(out_ap=rbc,
                                                          in_ap=rec)
                            nc.vector.tensor_tensor(
                                out=attnT[hp][j][64 * par:64 * par + 64, :],
                                in0=pvs[0:64, :], in1=rbc, op=AOT.mult)

                    oproj(j)
    nc.compile()
    return nc


def _get_program(causal: bool):
    key = ("v1", causal)
    if key not in _PROGRAM_CACHE:
        _PROGRAM_CACHE[key] = _build_program(causal)
    return _PROGRAM_CACHE[key]


def _check_causal(mask: np.ndarray) -> bool:
    m = mask.reshape(SEQ, SEQ)
    # spot-check pattern: 0 on/below diagonal, very negative above
    idx = np.array([0, 1, 7, 100, 1000, 2047])
    sub = m[np.ix_(idx, idx)]
    expect_zero = idx[:, None] >= idx[None, :]
    if not np.all(sub[expect_zero] == 0.0):
        return False
    if not np.all(sub[~expect_zero] < -1e30):
        return False
    return True


def kernel(x, Wq, Wk, Wv, Wo, cos, sin, attention_mask):
    from concourse.bass_utils import run_bass_kernel_spmd

    x = np.asarray(x, dtype=np.float32)
    Wq = np.asarray(Wq, dtype=np.float32)
    Wk = np.asarray(Wk, dtype=np.float32)
    Wv = np.asarray(Wv, dtype=np.float32)
    Wo = np.asarray(Wo, dtype=np.float32)
    cos = np.asarray(cos, dtype=np.float32)
    sin = np.asarray(sin, dtype=np.float32)
    mask = np.asarray(attention_mask, dtype=np.float32)

    causal = _check_causal(mask)
    if not causal:
        # fall back to dense attention with no masking only if mask is all 0
        assert np.all(mask == 0.0), (
            "kernel only supports the causal or all-zero attention masks")

    # host-preprocessed RoPE tables: transposed, duplicated to 128 partitions,
    # sign folded into sin for the rotate_half shift
    cosT = np.ascontiguousarray(cos.T)  # (64, SEQ)
    sinT = sin.T
    sin_signed = np.concatenate([-sinT[:32], sinT[32:]], axis=0)
    cos2 = np.ascontiguousarray(np.tile(cosT, (2, 1)))  # (128, SEQ)
    sin2 = np.ascontiguousarray(np.tile(sin_signed, (2, 1)))

    nc = _get_program(causal)

    in_maps = []
    for core in range(N_CORES):
        b, g4 = core // 4, core % 4
        in_maps.append({
            "xT": np.ascontiguousarray(x[b].T.astype(np.float16)),
            "wq": np.ascontiguousarray(
                Wq[:, g4 * DQ:(g4 + 1) * DQ].astype(np.float16)),
            "wk": np.ascontiguousarray(
                Wk[:, g4 * DKV:(g4 + 1) * DKV].astype(np.float16)),
            "wv": np.ascontiguousarray(
                Wv[:, g4 * DKV:(g4 + 1) * DKV].astype(np.float16)),
            "wo": np.ascontiguousarray(
                Wo[g4 * DQ:(g4 + 1) * DQ, :].astype(np.float16)),
            "cos2": cos2.astype(np.float16),
            "sin2": sin2.astype(np.float16),
        })

    trace = bool(int(os.environ.get("KERNEL_TRACE", "0")))
    res = run_bass_kernel_spmd(nc, in_maps, list(range(N_CORES)), trace=trace)
    if trace:
        kernel.last_exec_time_ns = res.exec_time_ns
        kernel.last_profile = res.profile_json

    outs = [res.results[i]["out"] for i in range(N_CORES)]
    y0 = outs[0] + outs[1] + outs[2] + outs[3]
    y1 = outs[4] + outs[5] + outs[6] + outs[7]
    return np.stack([y0, y1]).astype(np.float32)



# revision 12
# speedup vs baseline: 1.3156x; 1.3156x over previous
"""Trainium2 Bass kernel for multi-head attention (GQA + RoPE + causal) — v2.

Problem shapes (hardcoded):
  x: (2, 2048, 2048)  Wq: (2048, 2048->512/core)  Wk/Wv: (2048, 512->128/core)
  Wo: (2048->512/core, 2048)  cos/sin: (2048, 64)  mask: causal (1,1,2048,2048)

Sharding: 8 cores = 2 batches (DP) x 4 head groups (TP).  Each core handles
one batch sample and 8 query heads (= 2 KV heads, each kept with its 4 query
heads).  Wo's input dim is sharded, so each core produces a partial
(2048, 2048) output; the host sums the 4 partials per batch (fp32).

v2 changes over the 438us baseline:
  - software pipelining: the QKV projection of seq-block b+1 and the output
    projection of block b-1 are interleaved (generator "filler" units) into
    the exp-bound attention inner loop of block b, so the tensor engine never
    idles waiting on the scalar engine and HAM stays warm.
  - scalar engine runs ONLY the softmax exps; all DMA triggers, psum drains
    and the denominator path moved to vector/gpsimd/sync queues.
  - causal trimming: diagonal k-tiles stream only the valid q-range
    [128*i, 512) through scores/exp/PV, and the triangle masking is a
    128-col affine_select on the vector engine (was full-tile on gpsimd).
  - host pre-tiles x/weights into partition-major [128, c, n] layouts so
    each input needs only 1-4 large DMAs instead of 16 per tensor.
  - the softmax denominator (ones-column PV row 64) is read straight from
    psum by reciprocal_approx_fast, no staging copy.
  - output staged and DMA'd as fp16 (halves write traffic; host sums fp32).
"""

import os
import sys
from collections import deque

import numpy as np

if "/opt/trn_rl_repo" not in sys.path:
    sys.path.insert(0, "/opt/trn_rl_repo")

SEQ = 2048
DIM = 2048
HEAD_DIM = 64
DQ = 512          # query dims per core (8 heads)
DKV = 128         # kv dims per core (2 kv heads)
SCALE = HEAD_DIM ** -0.5
N_CORES = 8
NB = SEQ // 512   # 4 seq blocks of 512
NKT = SEQ // 128  # 16 k-tiles of 128

_PROGRAM_CACHE = {}


class _Filler:
    """FIFO of generator units; attention loops pump a few steps per
    iteration to fill tensor-engine slack while the scalar engine exps."""

    def __init__(self):
        self.q = deque()
        self.done = set()

    def add(self, label, gen_fn):
        self.q.append((label, gen_fn()))

    def step(self, n=1):
        while n > 0 and self.q:
            label, g = self.q[0]
            try:
                next(g)
                n -= 1
            except StopIteration:
                self.done.add(label)
                self.q.popleft()

    def run_until(self, label):
        while label not in self.done and self.q:
            lab, g = self.q[0]
            for _ in g:
                pass
            self.done.add(lab)
            self.q.popleft()

    def run_all(self):
        while self.q:
            lab, g = self.q[0]
            for _ in g:
                pass
            self.done.add(lab)
            self.q.popleft()


def _build_program(causal: bool):
    import concourse.bass as bass  # noqa: F401
    import concourse.mybir as mybir
    from concourse import bacc
    from concourse.masks import make_identity
    from concourse.tile import TileContext

    f32 = mybir.dt.float32
    f16 = mybir.dt.float16
    AOT = mybir.AluOpType
    EXP = mybir.ActivationFunctionType.Exp

    nc = bacc.Bacc(None, target_bir_lowering=False)
    # host-pretiled partition-major layouts
    xT2 = nc.declare_dram_parameter("xT2", [128, 16, SEQ], f16, isOutput=False)
    wq2 = nc.declare_dram_parameter("wq2", [128, 16, DQ], f16, isOutput=False)
    wk2 = nc.declare_dram_parameter("wk2", [128, 16, DKV], f16, isOutput=False)
    wv2 = nc.declare_dram_parameter("wv2", [128, 16, DKV], f16, isOutput=False)
    wo2 = nc.declare_dram_parameter("wo2", [128, 4, DIM], f16, isOutput=False)
    cos2 = nc.declare_dram_parameter("cos2", [128, SEQ], f16, isOutput=False)
    sin2 = nc.declare_dram_parameter("sin2", [128, SEQ], f16, isOutput=False)
    outp = nc.declare_dram_parameter("out", [SEQ, DIM], f16, isOutput=True)

    with TileContext(nc) as tc:
        with tc.tile_pool(name="pa", bufs=1) as pa, \
             tc.tile_pool(name="pw", bufs=1) as pw, \
             tc.tile_pool(name="pp", bufs=1, space="PSUM") as pp:

            # ---------------- persistent SBUF tiles ----------------
            wq_sb = pa.tile([128, 16, DQ], f16, name="wq_sb", tag="wq_sb")
            wk_sb = pa.tile([128, 16, DKV], f16, name="wk_sb", tag="wk_sb")
            wv_sb = pa.tile([128, 16, DKV], f16, name="wv_sb", tag="wv_sb")
            wo_sb = pa.tile([128, 4, DIM], f16, name="wo_sb", tag="wo_sb")
            cos_sb = pa.tile([128, SEQ], f16, name="cos_sb", tag="cos_sb")
            sin_sb = pa.tile([128, SEQ], f16, name="sin_sb", tag="sin_sb")
            identity = pa.tile([128, 128], f16, name="identity", tag="identity")
            qt = [[pa.tile([128, 512], f16, name=f"qt{hp}_{b}",
                           tag=f"qt{hp}_{b}") for b in range(NB)]
                  for hp in range(4)]
            ktr = [pa.tile([128, 512], f16, name=f"ktr{b}", tag=f"ktr{b}")
                   for b in range(NB)]
            kdup = [[pa.tile([128, 512], f16, name=f"kdup{g}_{b}",
                             tag=f"kdup{g}_{b}") for b in range(NB)]
                    for g in range(2)]
            # [V0 | 1 | V1 | 1] — ones column appended per kv head makes PV
            # row 64 the softmax denominator for free
            vtiles = [pa.tile([128, 130], f16, name=f"vt{i}", tag=f"vt{i}")
                      for i in range(NKT)]
            attnT = [[pa.tile([128, 512], f16, name=f"attnT{hp}_{b}",
                              tag=f"attnT{hp}_{b}") for b in range(NB)]
                     for hp in range(4)]

            # ---------------- initial DMAs (spread across queues) ----------
            # first-needed first; 4 queues pull in parallel
            for h in range(4):
                nc.gpsimd.dma_start(out=wk_sb[:, 4 * h:4 * h + 4, :],
                                    in_=wk2[:, 4 * h:4 * h + 4, :])
            nc.scalar.dma_start(out=cos_sb, in_=cos2[:, :])
            nc.scalar.dma_start(out=sin_sb, in_=sin2[:, :])
            for h in range(4):
                nc.scalar.dma_start(out=wq_sb[:, 4 * h:4 * h + 4, :],
                                    in_=wq2[:, 4 * h:4 * h + 4, :])
            nc.scalar.dma_start(out=wv_sb, in_=wv2[:, :, :])
            nc.scalar.dma_start(out=wo_sb, in_=wo2[:, :, :])
            make_identity(nc, identity)

            xt_tiles = {}
            filler = _Filler()

            # ---------------- pipeline units ----------------
            def rope_steps(chunk, b):
                # in-place RoPE on a [128, 512] Q^T/K^T chunk of seq block b
                sl = slice(b * 512, (b + 1) * 512)
                rot = pw.tile([128, 512], f16, name="rot", tag="rot", bufs=3)
                for blk in (0, 64):
                    nc.sync.dma_start(out=rot[blk:blk + 32, :],
                                      in_=chunk[blk + 32:blk + 64, :])
                    nc.sync.dma_start(out=rot[blk + 32:blk + 64, :],
                                      in_=chunk[blk:blk + 32, :])
                yield
                nc.vector.tensor_tensor(out=rot, in0=rot, in1=sin_sb[:, sl],
                                        op=AOT.mult)
                yield
                nc.vector.tensor_tensor(out=chunk, in0=chunk,
                                        in1=cos_sb[:, sl], op=AOT.mult)
                nc.vector.tensor_add(out=chunk, in0=chunk, in1=rot)
                yield

            def xt_unit(b):
                def gen():
                    t = pw.tile([128, 16, 512], f16, name=f"xt{b}", tag="xt",
                                bufs=2)
                    xt_tiles[b] = t
                    step = 2 if b == 0 else 4
                    eng = nc.sync if b % 2 == 0 else nc.gpsimd
                    for ci in range(0, 16, step):
                        eng.dma_start(
                            out=t[:, ci:ci + step, :],
                            in_=xT2[:, ci:ci + step, 512 * b:512 * b + 512])
                        yield
                return gen

            def proj_unit(b, kind, t=None):
                def gen():
                    xt = xt_tiles[b]
                    ps = pp.tile([128, 512], f32, name="ps", tag="qkvps",
                                 bufs=2)
                    for c in range(16):
                        if kind == "q":
                            lhsT = wq_sb[:, c, 128 * t:128 * t + 128]
                        elif kind == "k":
                            lhsT = wk_sb[:, c, :]
                        else:
                            lhsT = wv_sb[:, c, :]
                        nc.tensor.matmul(ps, lhsT=lhsT, rhs=xt[:, c, :],
                                         start=(c == 0), stop=(c == 15))
                        if c % 2 == 1:
                            yield
                    if kind == "q":
                        dst = qt[t][b]
                        nc.vector.tensor_copy(out=dst, in_=ps)
                        yield
                        yield from rope_steps(dst, b)
                    elif kind == "k":
                        dst = ktr[b]
                        nc.vector.tensor_copy(out=dst, in_=ps)
                        yield
                        yield from rope_steps(dst, b)
                        nc.gpsimd.dma_start(out=kdup[0][b][0:64, :],
                                            in_=dst[0:64, :])
                        nc.gpsimd.dma_start(out=kdup[0][b][64:128, :],
                                            in_=dst[0:64, :])
                        nc.gpsimd.dma_start(out=kdup[1][b][0:64, :],
                                            in_=dst[64:128, :])
                        nc.gpsimd.dma_start(out=kdup[1][b][64:128, :],
                                            in_=dst[64:128, :])
                        yield
                    else:
                        vtr = pw.tile([128, 512], f16, name="vtr", tag="vtr",
                                      bufs=2)
                        nc.vector.tensor_copy(out=vtr, in_=ps)
                        yield
                        for ii in range(4):
                            vp = pp.tile([128, 128], f16, name="vt_ps",
                                         tag="qkvps", bufs=2)
                            nc.tensor.transpose(
                                vp, vtr[:, 128 * ii:128 * ii + 128], identity)
                            i = 4 * b + ii
                            nc.vector.tensor_copy(out=vtiles[i][:, 0:64],
                                                  in_=vp[:, 0:64])
                            nc.vector.tensor_copy(out=vtiles[i][:, 65:129],
                                                  in_=vp[:, 64:128])
                            nc.vector.memset(vtiles[i][:, 64:65], 1.0)
                            nc.vector.memset(vtiles[i][:, 129:130], 1.0)
                            yield
                return gen

            def oproj_unit(b):
                def gen():
                    for s_ in range(4 * b, 4 * b + 4):
                        so = (s_ - 4 * b) * 128
                        ostage = pw.tile([128, DIM], f16, name="ostage",
                                         tag="ostage", bufs=2)
                        for dm in range(4):
                            ops = pp.tile([128, 512], f32, name="ops",
                                          tag="qkvps", bufs=2)
                            for c in range(4):
                                nc.tensor.matmul(
                                    ops, lhsT=attnT[c][b][:, so:so + 128],
                                    rhs=wo_sb[:, c, dm * 512:(dm + 1) * 512],
                                    start=(c == 0), stop=(c == 3))
                                if c % 2 == 1:
                                    yield
                            nc.vector.tensor_copy(
                                out=ostage[:, dm * 512:(dm + 1) * 512],
                                in_=ops)
                            yield
                        nc.sync.dma_start(
                            out=outp[128 * s_:128 * s_ + 128, :], in_=ostage)
                        yield
                return gen

            def emit_qkv(b):
                filler.add(f"xt@{b}", xt_unit(b))
                filler.add(f"k@{b}", proj_unit(b, "k"))
                filler.add(f"q0@{b}", proj_unit(b, "q", 0))
                filler.add(f"v@{b}", proj_unit(b, "v"))
                filler.add(f"q1@{b}", proj_unit(b, "q", 1))
                filler.add(f"q2@{b}", proj_unit(b, "q", 2))
                filler.add(f"q3@{b}", proj_unit(b, "q", 3))

            # ---------------- attention for one seq block ----------------
            def attention(b):
                nkt = 4 * b + 4 if causal else NKT
                filler.run_until(f"q0@{b}")
                filler.run_until(f"v@{b}")
                for hp in range(4):
                    filler.run_until(f"q{hp}@{b}")
                    g = hp // 2  # local kv head shared by the pair
                    pv = [pp.tile([65, 512], f32, name=f"pv{par}", tag="pv",
                                  bufs=2) for par in range(2)]
                    for kt in range(nkt):
                        i = kt - 4 * b  # diagonal offset (>=0 on/after diag)
                        lo = 128 * i if (causal and i >= 0) else 0
                        stt = pp.tile([128, 2, 512], f32, name="stt",
                                      tag="stt", bufs=2)
                        lk = kdup[g][kt // 4]
                        ck = slice((kt % 4) * 128, (kt % 4 + 1) * 128)
                        for h in (0, 1):
                            nc.tensor.matmul(
                                stt[:, h, lo:512],
                                lhsT=lk[64 * h:64 * h + 64, ck],
                                rhs=qt[hp][b][64 * h:64 * h + 64, lo:512],
                                start=True, stop=True,
                                tile_position=(64 * h, 0))
                        pt = pw.tile([128, 2, 512], f16, name="pt", tag="pt",
                                     bufs=6)
                        nc.scalar.activation(out=pt[:, :, lo:512],
                                             in_=stt[:, :, lo:512],
                                             func=EXP, scale=SCALE)
                        if causal and i >= 0:
                            # zero the strictly-above-diagonal triangle
                            nc.gpsimd.affine_select(
                                out=pt[:, :, lo:lo + 128],
                                in_=pt[:, :, lo:lo + 128],
                                pattern=[[0, 2], [1, 128]],
                                compare_op=AOT.is_ge,
                                fill=0.0, base=0, channel_multiplier=-1)
                        st, sp = (kt == 0), (kt == nkt - 1)
                        for h in (0, 1):
                            nc.tensor.matmul(
                                pv[h][:, lo:512],
                                lhsT=vtiles[kt][:, 65 * g:65 * g + 65],
                                rhs=pt[:, h, lo:512], start=st, stop=sp)
                        filler.step(1)
                    for par in (0, 1):
                        den = pw.tile([1, 512], f32, name="den", tag="den",
                                      bufs=4)
                        nc.scalar.copy(out=den, in_=pv[par][64:65, :])
                        rec = pw.tile([1, 512], f32, name="rec", tag="rec",
                                      bufs=4)
                        nc.vector.reciprocal_approx_fast(
                            out=rec, in_=den)
                        rbc = pw.tile([64, 512], f32, name="rbc", tag="rbc",
                                      bufs=4)
                        nc.gpsimd.partition_broadcast(out_ap=rbc, in_ap=rec)
                        nc.vector.tensor_tensor(
                            out=attnT[hp][b][64 * par:64 * par + 64, :],
                            in0=pv[par][0:64, :], in1=rbc, op=AOT.mult)
                    filler.step(2)

            # ---------------- main pipeline ----------------
            if causal:
                emit_qkv(0)
                for b in range(NB):
                    if b + 1 < NB:
                        emit_qkv(b + 1)
                    if b - 1 >= 0:
                        filler.add(f"oproj@{b - 1}", oproj_unit(b - 1))
                    attention(b)
                filler.run_all()
                for _ in oproj_unit(NB - 1)():
                    pass
            else:
                for b in range(NB):
                    emit_qkv(b)
                filler.run_all()
                for b in range(NB):
                    if b - 1 >= 0:
                        filler.add(f"oproj@{b - 1}", oproj_unit(b - 1))
                    attention(b)
                filler.run_all()
                for _ in oproj_unit(NB - 1)():
                    pass

    nc.compile()
    return nc


def _get_program(causal: bool):
    key = ("v2", causal)
    if key not in _PROGRAM_CACHE:
        _PROGRAM_CACHE[key] = _build_program(causal)
    return _PROGRAM_CACHE[key]


def _check_causal(mask: np.ndarray) -> bool:
    m = mask.reshape(SEQ, SEQ)
    idx = np.array([0, 1, 7, 100, 1000, 2047])
    sub = m[np.ix_(idx, idx)]
    expect_zero = idx[:, None] >= idx[None, :]
    if not np.all(sub[expect_zero] == 0.0):
        return False
    if not np.all(sub[~expect_zero] < -1e30):
        return False
    return True


def _tile_pm(a: np.ndarray, nchunk: int) -> np.ndarray:
    """[nchunk*128, n] -> partition-major [128, nchunk, n] fp16."""
    n = a.shape[1]
    return np.ascontiguousarray(
        a.reshape(nchunk, 128, n).transpose(1, 0, 2).astype(np.float16))


def kernel(x, Wq, Wk, Wv, Wo, cos, sin, attention_mask):
    from concourse.bass_utils import run_bass_kernel_spmd

    x = np.asarray(x, dtype=np.float32)
    Wq = np.asarray(Wq, dtype=np.float32)
    Wk = np.asarray(Wk, dtype=np.float32)
    Wv = np.asarray(Wv, dtype=np.float32)
    Wo = np.asarray(Wo, dtype=np.float32)
    cos = np.asarray(cos, dtype=np.float32)
    sin = np.asarray(sin, dtype=np.float32)
    mask = np.asarray(attention_mask, dtype=np.float32)

    causal = _check_causal(mask)
    if not causal:
        assert np.all(mask == 0.0), (
            "kernel only supports the causal or all-zero attention masks")

    # host-preprocessed RoPE tables: transposed, duplicated to 128 partitions,
    # sign folded into sin for the rotate_half shift
    cosT = np.ascontiguousarray(cos.T)  # (64, SEQ)
    sinT = sin.T
    sin_signed = np.concatenate([-sinT[:32], sinT[32:]], axis=0)
    cos2 = np.ascontiguousarray(np.tile(cosT, (2, 1))).astype(np.float16)
    sin2 = np.ascontiguousarray(np.tile(sin_signed, (2, 1))).astype(np.float16)

    nc = _get_program(causal)

    in_maps = []
    for core in range(N_CORES):
        b, g4 = core // 4, core % 4
        in_maps.append({
            "xT2": _tile_pm(np.ascontiguousarray(x[b].T), 16),
            "wq2": _tile_pm(Wq[:, g4 * DQ:(g4 + 1) * DQ], 16),
            "wk2": _tile_pm(Wk[:, g4 * DKV:(g4 + 1) * DKV], 16),
            "wv2": _tile_pm(Wv[:, g4 * DKV:(g4 + 1) * DKV], 16),
            "wo2": _tile_pm(Wo[g4 * DQ:(g4 + 1) * DQ, :], 4),
            "cos2": cos2,
            "sin2": sin2,
        })

    trace = bool(int(os.environ.get("KERNEL_TRACE", "0")))
    res = run_bass_kernel_spmd(nc, in_maps, list(range(N_CORES)), trace=trace)
    if trace:
        kernel.last_exec_time_ns = res.exec_time_ns
        kernel.last_profile = res.profile_json

    outs = [res.results[i]["out"].astype(np.float32) for i in range(N_CORES)]
    y0 = outs[0] + outs[1] + outs[2] + outs[3]
    y1 = outs[4] + outs[5] + outs[6] + outs[7]
    return np.stack([y0, y1]).astype(np.float32)
